# revision 34
# baseline (speedup 1.0000x reference)
"""Trainium2 Bass kernel for the dual-stream encoder block.

Sharding: 8 cores = 4 batches x 2 query-row halves (2048 rows/core).
Inputs are pre-rolled along L per core so output rows are always 0..2047;
K/V contraction uses the full 4096 rows. No cross-core communication.
"""

import sys

sys.path.insert(0, "/opt/trn_rl_repo")

import numpy as np
import ml_dtypes

B, L, D, OUT = 4, 4096, 128, 55
D2, H = 256, 512
A = 2048  # output rows per core
NT = 32  # l-tiles of 128
NG = 8  # l-groups of 4 tiles
AT = 16  # a-tiles per core
AC = 4  # a-chunks of 512
SCALE = float(1.0 / np.sqrt(np.float32(128.0)))

_CACHE = {}


def _build_nc():
    import concourse.bass as bass
    from concourse import bacc, mybir
    import concourse.tile as tile
    from concourse.masks import make_identity

    f32 = mybir.dt.float32
    bf16 = mybir.dt.bfloat16
    AF = mybir.ActivationFunctionType
    ALU = mybir.AluOpType

    nc = bacc.Bacc("TRN2", target_bir_lowering=False, debug=False)

    # ---- DRAM parameters -------------------------------------------------
    dx1 = nc.dram_tensor("x1", [128, NT, D], f32, kind="ExternalInput")
    dx2 = nc.dram_tensor("x2", [128, NT, D], f32, kind="ExternalInput")
    dres1 = nc.dram_tensor("res1p", [128, AT, D], f32, kind="ExternalInput")
    dres2 = nc.dram_tensor("res2p", [128, AT, D], f32, kind="ExternalInput")
    dwpack = nc.dram_tensor("wpack", [128, 2926], bf16, kind="ExternalInput")
    dvpack = nc.dram_tensor("vpack", [128, 6], f32, kind="ExternalInput")
    dbf2 = nc.dram_tensor("bf2", [D2], f32, kind="ExternalInput")
    dbo = nc.dram_tensor("bo", [OUT], f32, kind="ExternalInput")
    dout = nc.dram_tensor("out", [128, AT, OUT], f32, kind="ExternalOutput")

    def bcast_ap(dt_handle, n):
        ap = dt_handle.ap()
        return bass.AP(tensor=ap.tensor, offset=ap.offset, ap=[[0, 128], [1, n]])

    with tile.TileContext(nc) as tc:
        import contextlib

        with contextlib.ExitStack() as ctx:
            consts = ctx.enter_context(tc.tile_pool(name="consts", bufs=1))
            big = ctx.enter_context(tc.tile_pool(name="big", bufs=1))
            stats = ctx.enter_context(tc.tile_pool(name="stats", bufs=3))
            xpool = ctx.enter_context(tc.tile_pool(name="xpool", bufs=2))
            xspool = ctx.enter_context(tc.tile_pool(name="xs", bufs=5))
            respool = ctx.enter_context(tc.tile_pool(name="res", bufs=1))
            outpool = ctx.enter_context(tc.tile_pool(name="outp", bufs=1))

            # ---- constants ----
            ident = consts.tile([128, 128], bf16)
            make_identity(nc, ident[:])
            ones_bf = consts.tile([128, 1], bf16)
            nc.vector.memset(ones_bf[:], 1.0)

            wpk = consts.tile([128, 2926], bf16)
            nc.gpsimd.dma_start(wpk[:], dwpack[:])
            vpk = consts.tile([128, 6], f32)
            nc.gpsimd.dma_start(vpk[:], dvpack[:])
            wq = wpk[:, 0:128]
            wk = wpk[:, 128:256]
            wv1 = wpk[:, 256:384]
            wv2 = wpk[:, 384:512]
            wp1 = wpk[:, 512:640]
            wp2 = wpk[:, 640:768]
            wf1v = lambda kh, n: wpk[:, 768 + 512 * kh + 128 * n : 768 + 512 * kh + 128 * (n + 1)]
            wf2v = lambda st: wpk[:, 1792 + 256 * st : 1792 + 256 * (st + 1)]
            wov = lambda sh: wpk[:, 2816 + 55 * sh : 2816 + 55 * (sh + 1)]
            bq = vpk[:, 0:1]
            bk = vpk[:, 1:2]
            bf1t = vpk[:, 2:6]
            bf2b = consts.tile([128, D2], f32)
            nc.gpsimd.dma_start(bf2b[:], bcast_ap(dbf2, D2))
            bob = consts.tile([128, OUT], f32)
            nc.gpsimd.dma_start(bob[:], bcast_ap(dbo, OUT))

            # ---- big SBUF residents ----
            x1nT = big.tile([128, L], bf16)  # [d, l]
            x2nT = big.tile([128, L], bf16)
            q1T = big.tile([128, L], bf16)
            k2T = big.tile([128, A], bf16)
            v1 = big.tile([128, NT * 128], bf16)  # [l-part, tile*d]
            v2 = big.tile([128, NT * 128], bf16)
            attT = big.tile([128, NT, 512], bf16)
            o1T = big.tile([128, A], bf16)  # unnormalized (att @ v1)^T
            o2T = big.tile([128, A], bf16)
            invd = big.tile([128, AT], f32)
            xcat = big.tile([128, AT, D2], f32)
            xfTl = big.tile([128, A], bf16)
            xfTh = big.tile([128, A], bf16)
            h1T = big.tile([128, 4, A], bf16)
            T8 = big.tile([128, 8, 512], bf16)
            attP = [big.tile([128, 4, 512], bf16, name="attP0", tag="attP0"), big.tile([128, 4, 512], bf16, name="attP1", tag="attP1")]

            # =========== Phase A: LN + transpose + QKV projections =======
            def stream_phase(dx, xnT, first):
                xv = dx.ap()
                BS = stats.tile([128, NT, 6], f32, tag="BS")
                MV = stats.tile([128, NT, 2], f32, tag="MV")
                IV = stats.tile([128, NT], f32, tag="IV")
                RS = stats.tile([128, NT], f32, tag="RS")
                for hh in range(2):
                    X = xpool.tile([128, 16, 128], f32, tag="X")
                    for gg_ in range(4):
                        g = 4 * hh + gg_
                        nc.sync.dma_start(
                            X[:, 4 * gg_ : 4 * gg_ + 4, :], xv[:, 4 * g : 4 * g + 4, :]
                        )
                        for k in range(4):
                            nc.vector.bn_stats(
                                BS[:, 4 * g + k, :], X[:, 4 * gg_ + k, :]
                            )
                    for gg_ in range(4):
                        g = 4 * hh + gg_
                        for k in range(4):
                            i = 4 * g + k
                            nc.vector.bn_aggr(MV[:, i, :], BS[:, i, :])
                        sl = slice(4 * g, 4 * (g + 1))
                        nc.vector.reciprocal(IV[:, sl], MV[:, sl, 1])
                        nc.scalar.activation(RS[:, sl], IV[:, sl], AF.Sqrt)
                    for gp in range(2):
                        psT = psA.tile([128, 1024], bf16, tag="tr")
                        for kk in range(8):
                            i = 16 * hh + 8 * gp + kk
                            xs = xspool.tile([128, 128], bf16, tag="xs")
                            nc.vector.tensor_scalar(
                                xs[:],
                                X[:, 8 * gp + kk, :],
                                MV[:, i, 0:1],
                                RS[:, i : i + 1],
                                op0=ALU.subtract,
                                op1=ALU.mult,
                            )
                            nc.tensor.transpose(
                                psT[:, 128 * kk : 128 * (kk + 1)], xs[:], ident[:]
                            )
                        nc.scalar.copy(
                            xnT[:, 1024 * (2 * hh + gp) : 1024 * (2 * hh + gp + 1)],
                            psT[:],
                        )

            psA_cm = tc.tile_pool(name="psA", bufs=2, space="PSUM")
            psA = psA_cm.__enter__()

            # stream 2 first (k2T is needed by every attention chunk)
            stream_phase(dx2, x2nT, True)
            # k2T chunks (a-cols 0..2047) + v2 tiles
            for c in range(4):
                ps = psA.tile([128, 512], f32, tag="qk")
                nc.tensor.matmul(
                    ps[:], wk, x2nT[:, 512 * c : 512 * (c + 1)], start=True, stop=True
                )
                nc.scalar.activation(
                    k2T[:, 512 * c : 512 * (c + 1)], ps[:], AF.Identity, bias=bk
                )
            for g in range(NG):
                psv = psA.tile([128, 512], f32, tag="v")
                for k in range(4):
                    i = 4 * g + k
                    nc.tensor.matmul(
                        psv[:, 128 * k : 128 * (k + 1)],
                        x2nT[:, 128 * i : 128 * (i + 1)],
                        wv2,
                        start=True,
                        stop=True,
                    )
                nc.scalar.copy(v2[:, 512 * g : 512 * (g + 1)], psv[:])

            stream_phase(dx1, x1nT, False)
            for c in range(NG):
                ps = psA.tile([128, 512], f32, tag="qk")
                nc.tensor.matmul(
                    ps[:], wq, x1nT[:, 512 * c : 512 * (c + 1)], start=True, stop=True
                )
                nc.scalar.activation(
                    q1T[:, 512 * c : 512 * (c + 1)], ps[:], AF.Identity, bias=bq
                )
            for g in range(NG):
                psv = psA.tile([128, 512], f32, tag="v")
                for k in range(4):
                    i = 4 * g + k
                    nc.tensor.matmul(
                        psv[:, 128 * k : 128 * (k + 1)],
                        x1nT[:, 128 * i : 128 * (i + 1)],
                        wv1,
                        start=True,
                        stop=True,
                    )
                nc.vector.tensor_copy(v1[:, 512 * g : 512 * (g + 1)], psv[:])

            psA_cm.__exit__(None, None, None)

            r1 = respool.tile([128, AT, D], f32, tag="r1")
            nc.sync.dma_start(r1[:], dres1.ap())
            r2 = respool.tile([128, AT, D], f32, tag="r2")
            nc.sync.dma_start(r2[:], dres2.ap())

            # =========== Phase B: attention =============================
            psB_cm = tc.tile_pool(name="psB", bufs=2, space="PSUM")
            psB = psB_cm.__enter__()
            psBo_cm = tc.tile_pool(name="psBo", bufs=1, space="PSUM")
            psBo = psBo_cm.__enter__()
            psBo1_cm = tc.tile_pool(name="psBo1", bufs=2, space="PSUM")
            psBo1 = psBo1_cm.__enter__()
            BSf = stats.tile([128, AT, 6], f32, tag="BSf")
            MVf = stats.tile([128, AT, 2], f32, tag="MVf")
            IVf = stats.tile([128, AT], f32, tag="IVf")
            RSf = stats.tile([128, AT], f32, tag="RSf")
            for j in range(AC):
                psO1 = psBo1.tile([128, 512], f32, tag="o1")
                psO2 = psBo.tile([128, 512], f32, tag="o2")
                def av_mms(g):
                    for k in range(2):
                        i = 2 * g + k
                        nc.tensor.matmul(
                            psO1[:],
                            v1[:, 128 * i : 128 * (i + 1)],
                            att_i(i),
                            start=(i == 0),
                            stop=(i == NT - 1),
                            skip_group_check=True,
                        )
                        nc.tensor.matmul(
                            psO2[:],
                            v2[:, 128 * i : 128 * (i + 1)],
                            att_i(i),
                            start=(i == 0),
                            stop=(i == NT - 1),
                            skip_group_check=True,
                        )

                aP = attP[j % 2]
                att_i = lambda i: aP[:, i, :] if i < 4 else attT[:, i, :]
                for g in range(NT // 2):
                    psE = psB.tile([128, 2, 512], f32, tag="e")
                    for k in range(2):
                        i = 2 * g + k
                        nc.tensor.matmul(
                            psE[:, k, :],
                            q1T[:, 128 * i : 128 * (i + 1)],
                            k2T[:, 512 * j : 512 * (j + 1)],
                            start=True,
                            stop=True,
                        )
                    if g < 2:
                        nc.scalar.activation(
                            aP[:, 2 * g : 2 * g + 2, :], psE[:], AF.Exp, scale=SCALE
                        )
                    else:
                        nc.scalar.activation(
                            attT[:, 2 * g : 2 * g + 2, :], psE[:], AF.Exp, scale=SCALE
                        )
                    if g > 0:
                        av_mms(g - 1)
                av_mms(NT // 2 - 1)
                # denominator: pairwise tree into scratch + ones matmuls
                nc.vector.tensor_tensor(
                    T8[:, 0:4, :], aP[:], attT[:, 8:12, :], op=ALU.add
                )
                nc.vector.tensor_tensor(
                    T8[:, 4:8, :], attT[:, 4:8, :], attT[:, 12:16, :], op=ALU.add
                )
                nc.vector.tensor_tensor(T8[:], T8[:], attT[:, 16:24, :], op=ALU.add)
                nc.vector.tensor_tensor(T8[:], T8[:], attT[:, 24:32, :], op=ALU.add)
                w = 8
                while w > 1:
                    w //= 2
                    nc.vector.tensor_tensor(
                        T8[:, 0:w, :], T8[:, 0:w, :], T8[:, w : 2 * w, :], op=ALU.add
                    )
                psDen = psBo.tile([128, 4], f32, tag="denp")
                for t in range(4):
                    nc.tensor.matmul(
                        psDen[:, t : t + 1],
                        T8[:, 0, 128 * t : 128 * (t + 1)],
                        ones_bf[:],
                        start=True,
                        stop=True,
                    )
                nc.vector.reciprocal(invd[:, 4 * j : 4 * j + 4], psDen[:])
                nc.scalar.copy(o1T[:, 512 * j : 512 * (j + 1)], psO1[:])
                nc.scalar.copy(o2T[:, 512 * j : 512 * (j + 1)], psO2[:])
                # inline output projection + residual + lnf stats for this chunk
                for k in range(4):
                    t = 4 * j + k
                    for (oT, wp, rr, off) in (
                        (o1T, wp1, r1[:, t, :], 0),
                        (o2T, wp2, r2[:, t, :], D),
                    ):
                        psP = psBo.tile([128, D], f32, tag="denp")
                        nc.tensor.matmul(
                            psP[:], oT[:, 128 * t : 128 * (t + 1)], wp,
                            start=True, stop=True,
                        )
                        sc = xspool.tile([128, D], f32, tag="sc")
                        nc.vector.tensor_scalar(
                            sc[:], psP[:], invd[:, t : t + 1], None, op0=ALU.mult
                        )
                        nc.gpsimd.tensor_tensor(
                            xcat[:, t, off : off + D], sc[:], rr, op=ALU.add
                        )
                    nc.vector.bn_stats(BSf[:, t, :], xcat[:, t, :])
                    nc.vector.bn_aggr(MVf[:, t, :], BSf[:, t, :])
                sl = slice(4 * j, 4 * (j + 1))
                nc.vector.reciprocal(IVf[:, sl], MVf[:, sl, 1])

            psBo1_cm.__exit__(None, None, None)
            psBo_cm.__exit__(None, None, None)
            psB_cm.__exit__(None, None, None)

            # =========== Phase D pools ==================================
            psC = ctx.enter_context(tc.tile_pool(name="psC", bufs=2, space="PSUM"))
            psH_pool = ctx.enter_context(tc.tile_pool(name="psH", bufs=1, space="PSUM"))

            # =========== Phase D: FFN + final LN + output ================
            def ln_to_T(src_xcat, dstl, dsth, MVx, RSx, chunks, act_evac=False):
                for jj in chunks:
                    psT2 = psC.tile([128, 4, 256], bf16, tag="tr2")
                    for k in range(4):
                        t = 4 * jj + k
                        xsf = xspool.tile([128, D2], bf16, tag="xsf")
                        nc.vector.tensor_scalar(
                            xsf[:],
                            src_xcat[:, t, :],
                            MVx[:, t, 0:1],
                            RSx[:, t : t + 1],
                            op0=ALU.subtract,
                            op1=ALU.mult,
                        )
                        nc.tensor.transpose(psT2[:, k, 0:128], xsf[:, 0:128], ident[:])
                        nc.tensor.transpose(psT2[:, k, 128:256], xsf[:, 128:256], ident[:])
                    evac = nc.scalar.copy if act_evac else nc.vector.tensor_copy
                    evac(dstl[:, 512 * jj : 512 * (jj + 1)], psT2[:, :, 0:128])
                    evac(dsth[:, 512 * jj : 512 * (jj + 1)], psT2[:, :, 128:256])

            BS3 = stats.tile([128, AT, 6], f32, tag="BS3")
            MV3 = stats.tile([128, AT, 2], f32, tag="MV3")
            IV3 = stats.tile([128, AT], f32, tag="IV3")
            RS3 = stats.tile([128, AT], f32, tag="RS3")
            nc.scalar.activation(RSf[:], IVf[:], AF.Sqrt)
            for jp in range(2):
                ln_to_T(xcat, xfTl, xfTh, MVf, RSf, [2 * jp, 2 * jp + 1])
                for n in range(4):
                    psH = psH_pool.tile([128, 2, 512], f32, tag="h")
                    for jj in range(2):
                        j = 2 * jp + jj
                        nc.tensor.matmul(
                            psH[:, jj, :],
                            wf1v(0, n),
                            xfTl[:, 512 * j : 512 * (j + 1)],
                            start=True,
                            stop=False,
                            skip_group_check=True,
                        )
                        nc.tensor.matmul(
                            psH[:, jj, :],
                            wf1v(1, n),
                            xfTh[:, 512 * j : 512 * (j + 1)],
                            start=False,
                            stop=True,
                            skip_group_check=True,
                        )
                    nc.scalar.activation(
                        h1T[:, n, 1024 * jp : 1024 * (jp + 1)],
                        psH[:],
                        AF.Gelu,
                        bias=bf1t[:, n : n + 1],
                    )
                for t in range(8 * jp, 8 * jp + 8):
                    psH2 = psC.tile([128, D2], f32, tag="h2")
                    for st in range(4):
                        nc.tensor.matmul(
                            psH2[:],
                            h1T[:, st, 128 * t : 128 * (t + 1)],
                            wf2v(st),
                            start=(st == 0),
                            stop=(st == 3),
                            skip_group_check=True,
                        )
                    sc = xspool.tile([128, D2], f32, tag="sc2")
                    nc.vector.tensor_tensor(sc[:], psH2[:], bf2b[:], op=ALU.add)
                    nc.gpsimd.tensor_tensor(
                        xcat[:, t, :], sc[:], xcat[:, t, :], op=ALU.add
                    )
                    nc.vector.bn_stats(BS3[:, t, :], xcat[:, t, :])
                    nc.vector.bn_aggr(MV3[:, t, :], BS3[:, t, :])
                    if t % 4 == 3:
                        nc.vector.reciprocal(
                            IV3[:, t - 3 : t + 1], MV3[:, t - 3 : t + 1, 1]
                        )

            # ln3 (reuse xfT buffers as x3T); sqrt batched to keep the gelu
            # table-set block contiguous
            nc.scalar.activation(RS3[:], IV3[:], AF.Sqrt)
            ov = dout.ap()
            osb = outpool.tile([128, AT, OUT], f32, tag="osb")
            for jj in range(AC):
                ln_to_T(xcat, xfTl, xfTh, MV3, RS3, [jj], act_evac=True)
                for t in range(4 * jj, 4 * jj + 4):
                    psO = psC.tile([128, OUT], f32, tag="p")
                    nc.tensor.matmul(
                        psO[:],
                        xfTl[:, 128 * t : 128 * (t + 1)],
                        wov(0),
                        start=True,
                        stop=False,
                        skip_group_check=True,
                    )
                    nc.tensor.matmul(
                        psO[:],
                        xfTh[:, 128 * t : 128 * (t + 1)],
                        wov(1),
                        start=False,
                        stop=True,
                        skip_group_check=True,
                    )
                    nc.vector.tensor_tensor(osb[:, t, :], psO[:], bob[:], op=ALU.add)
                nc.sync.dma_start(ov[:, 4 * jj : 4 * jj + 4, :], osb[:, 4 * jj : 4 * jj + 4, :])

    nc.compile()
    return nc


def _get_nc():
    if "nc" not in _CACHE:
        _CACHE["nc"] = _build_nc()
    return _CACHE["nc"]


def kernel(**inputs):
    from concourse.bass_utils import run_bass_kernel_spmd

    f = lambda k: np.asarray(inputs[k], dtype=np.float32)
    bf = lambda a: np.asarray(a, dtype=np.float32).astype(ml_dtypes.bfloat16)

    x1, x2 = f("x1"), f("x2")
    g1, b1 = f("ln1_g"), f("ln1_b")
    g2, b2 = f("ln2_g"), f("ln2_b")
    gf_, bf_ = f("lnf_g"), f("lnf_b")
    g3, b3 = f("ln3_g"), f("ln3_b")
    # fold LN gains/biases into the adjacent linear layers
    Wq = g1[:, None] * f("Wq"); bqp = b1 @ f("Wq") + f("bq")
    Wk = g2[:, None] * f("Wk"); bkp = b2 @ f("Wk") + f("bk")
    Wv1 = g1[:, None] * f("Wv1"); bv1p = b1 @ f("Wv1") + f("bv1")
    Wv2 = g2[:, None] * f("Wv2"); bv2p = b2 @ f("Wv2") + f("bv2")
    Wf1 = gf_[:, None] * f("Wf1"); bf1p = bf_ @ f("Wf1") + f("bf1")
    Wo = g3[:, None] * f("Wo"); bop = b3 @ f("Wo") + f("bo")
    Wp1, Wp2 = f("Wp1"), f("Wp2")
    bp1p = bv1p @ Wp1 + f("bp1")
    bp2p = bv2p @ Wp2 + f("bp2")

    Wf2 = f("Wf2")
    wpack = np.concatenate(
        [bf(Wq), bf(Wk), bf(Wv1), bf(Wv2), bf(Wp1), bf(Wp2),
         # Wf1 [256,512] -> [128, 2*4*128] as (kp, kh, n, np)
         bf(Wf1).reshape(2, 128, 4, 128).transpose(1, 0, 2, 3).reshape(128, 1024),
         # Wf2 [512,256] -> [128, 4*256] as (p, s, n)
         bf(Wf2).reshape(4, 128, D2).transpose(1, 0, 2).reshape(128, 1024),
         # Wo [256,55] -> [128, 2*55]
         bf(Wo).reshape(2, 128, OUT).transpose(1, 0, 2).reshape(128, 2 * OUT)],
        axis=1,
    )
    vpack = np.concatenate(
        [bqp.reshape(1, D), bkp.reshape(1, D), bf1p.reshape(4, D)], axis=0
    ).T.astype(np.float32)
    shared = {
        "wpack": np.ascontiguousarray(wpack),
        "vpack": np.ascontiguousarray(vpack),
        "bf2": f("bf2"), "bo": bop,
    }

    in_maps = []
    for c in range(8):
        b, h = c // 2, c % 2
        if h == 0:
            x1c, x2c = x1[b], x2[b]
        else:
            x1c = np.concatenate([x1[b, A:], x1[b, :A]], axis=0)
            x2c = np.concatenate([x2[b, A:], x2[b, :A]], axis=0)
        tilep = lambda M, nt: np.ascontiguousarray(
            M.reshape(nt, 128, D).transpose(1, 0, 2)
        )
        m = dict(shared)
        m["x1"] = tilep(x1c, NT)
        m["x2"] = tilep(x2c, NT)
        m["res1p"] = tilep(x1c[:A] + bp1p, AT)
        m["res2p"] = tilep(x2c[:A] + bp2p, AT)
        in_maps.append(m)

    nc = _get_nc()
    res = run_bass_kernel_spmd(nc, in_maps, core_ids=list(range(8)))
    out = np.empty((B, L, OUT), np.float32)
    for c in range(8):
        b, h = c // 2, c % 2
        oc = res.results[c]["out"].transpose(1, 0, 2).reshape(A, OUT)
        out[b, h * A : (h + 1) * A, :] = oc
    return out


# revision 35
# speedup vs baseline: 1.0116x; 1.0116x over previous
"""Trainium2 Bass kernel for the dual-stream encoder block.

Sharding: 8 cores = 4 batches x 2 query-row halves (2048 rows/core).
Inputs are pre-rolled along L per core so output rows are always 0..2047;
K/V contraction uses the full 4096 rows. No cross-core communication.
"""

import sys

sys.path.insert(0, "/opt/trn_rl_repo")

import numpy as np
import ml_dtypes

B, L, D, OUT = 4, 4096, 128, 55
D2, H = 256, 512
A = 2048  # output rows per core
NT = 32  # l-tiles of 128
NG = 8  # l-groups of 4 tiles
AT = 16  # a-tiles per core
AC = 4  # a-chunks of 512
SCALE = float(1.0 / np.sqrt(np.float32(128.0)))

_CACHE = {}


def _build_nc():
    import concourse.bass as bass
    from concourse import bacc, mybir
    import concourse.tile as tile
    from concourse.masks import make_identity

    f32 = mybir.dt.float32
    bf16 = mybir.dt.bfloat16
    AF = mybir.ActivationFunctionType
    ALU = mybir.AluOpType

    nc = bacc.Bacc("TRN2", target_bir_lowering=False, debug=False)

    # ---- DRAM parameters -------------------------------------------------
    dx1 = nc.dram_tensor("x1", [128, NT, D], f32, kind="ExternalInput")
    dx2 = nc.dram_tensor("x2", [128, NT, D], f32, kind="ExternalInput")
    dres1 = nc.dram_tensor("res1p", [128, AT, D], f32, kind="ExternalInput")
    dres2 = nc.dram_tensor("res2p", [128, AT, D], f32, kind="ExternalInput")
    dwpack = nc.dram_tensor("wpack", [128, 2926], bf16, kind="ExternalInput")
    dvpack = nc.dram_tensor("vpack", [128, 6], f32, kind="ExternalInput")
    dbf2 = nc.dram_tensor("bf2", [D2], f32, kind="ExternalInput")
    dbo = nc.dram_tensor("bo", [OUT], f32, kind="ExternalInput")
    dout = nc.dram_tensor("out", [128, AT, OUT], f32, kind="ExternalOutput")

    def bcast_ap(dt_handle, n):
        ap = dt_handle.ap()
        return bass.AP(tensor=ap.tensor, offset=ap.offset, ap=[[0, 128], [1, n]])

    with tile.TileContext(nc) as tc:
        import contextlib

        with contextlib.ExitStack() as ctx:
            consts = ctx.enter_context(tc.tile_pool(name="consts", bufs=1))
            big = ctx.enter_context(tc.tile_pool(name="big", bufs=1))
            stats = ctx.enter_context(tc.tile_pool(name="stats", bufs=3))
            xpool = ctx.enter_context(tc.tile_pool(name="xpool", bufs=2))
            xspool = ctx.enter_context(tc.tile_pool(name="xs", bufs=5))
            respool = ctx.enter_context(tc.tile_pool(name="res", bufs=1))
            outpool = ctx.enter_context(tc.tile_pool(name="outp", bufs=1))

            # ---- constants ----
            ident = consts.tile([128, 128], bf16)
            make_identity(nc, ident[:])
            ones_bf = consts.tile([128, 1], bf16)
            nc.vector.memset(ones_bf[:], 1.0)

            wpk = consts.tile([128, 2926], bf16)
            nc.gpsimd.dma_start(wpk[:], dwpack[:])
            vpk = consts.tile([128, 6], f32)
            nc.gpsimd.dma_start(vpk[:], dvpack[:])
            wq = wpk[:, 0:128]
            wk = wpk[:, 128:256]
            wv1 = wpk[:, 256:384]
            wv2 = wpk[:, 384:512]
            wp1 = wpk[:, 512:640]
            wp2 = wpk[:, 640:768]
            wf1v = lambda kh, n: wpk[:, 768 + 512 * kh + 128 * n : 768 + 512 * kh + 128 * (n + 1)]
            wf2v = lambda st: wpk[:, 1792 + 256 * st : 1792 + 256 * (st + 1)]
            wov = lambda sh: wpk[:, 2816 + 55 * sh : 2816 + 55 * (sh + 1)]
            bq = vpk[:, 0:1]
            bk = vpk[:, 1:2]
            bf1t = vpk[:, 2:6]
            bf2b = consts.tile([128, D2], f32)
            nc.gpsimd.dma_start(bf2b[:], bcast_ap(dbf2, D2))
            bob = consts.tile([128, OUT], f32)
            nc.gpsimd.dma_start(bob[:], bcast_ap(dbo, OUT))

            # ---- big SBUF residents ----
            x1nT = big.tile([128, L], bf16)  # [d, l]
            x2nT = big.tile([128, L], bf16)
            q1T = big.tile([128, L], bf16)
            k2T = big.tile([128, A], bf16)
            v1 = big.tile([128, NT * 128], bf16)  # [l-part, tile*d]
            v2 = big.tile([128, NT * 128], bf16)
            attT = big.tile([128, NT, 512], bf16)
            o1T = big.tile([128, A], bf16)  # unnormalized (att @ v1)^T
            o2T = big.tile([128, A], bf16)
            invd = big.tile([128, AT], f32)
            xcat = big.tile([128, AT, D2], f32)
            xfTl = big.tile([128, A], bf16)
            xfTh = big.tile([128, A], bf16)
            h1T = big.tile([128, 4, A], bf16)
            T8 = big.tile([128, 8, 512], bf16)
            attP = [big.tile([128, 4, 512], bf16, name="attP0", tag="attP0"), big.tile([128, 4, 512], bf16, name="attP1", tag="attP1")]

            # =========== Phase A: LN + transpose + QKV projections =======
            def stream_phase(dx, xnT, first):
                xv = dx.ap()
                BS = stats.tile([128, NT, 6], f32, tag="BS")
                MV = stats.tile([128, NT, 2], f32, tag="MV")
                IV = stats.tile([128, NT], f32, tag="IV")
                RS = stats.tile([128, NT], f32, tag="RS")
                for hh in range(2):
                    X = xpool.tile([128, 16, 128], f32, tag="X")
                    for gg_ in range(4):
                        g = 4 * hh + gg_
                        nc.sync.dma_start(
                            X[:, 4 * gg_ : 4 * gg_ + 4, :], xv[:, 4 * g : 4 * g + 4, :]
                        )
                        for k in range(4):
                            nc.vector.bn_stats(
                                BS[:, 4 * g + k, :], X[:, 4 * gg_ + k, :]
                            )
                    for gg_ in range(4):
                        g = 4 * hh + gg_
                        for k in range(4):
                            i = 4 * g + k
                            nc.vector.bn_aggr(MV[:, i, :], BS[:, i, :])
                        sl = slice(4 * g, 4 * (g + 1))
                        nc.vector.reciprocal(IV[:, sl], MV[:, sl, 1])
                        nc.scalar.activation(RS[:, sl], IV[:, sl], AF.Sqrt)
                    for gp in range(2):
                        psT = psA.tile([128, 1024], bf16, tag="tr")
                        for kk in range(8):
                            i = 16 * hh + 8 * gp + kk
                            xs = xspool.tile([128, 128], bf16, tag="xs")
                            nc.vector.tensor_scalar(
                                xs[:],
                                X[:, 8 * gp + kk, :],
                                MV[:, i, 0:1],
                                RS[:, i : i + 1],
                                op0=ALU.subtract,
                                op1=ALU.mult,
                            )
                            nc.tensor.transpose(
                                psT[:, 128 * kk : 128 * (kk + 1)], xs[:], ident[:]
                            )
                        nc.scalar.copy(
                            xnT[:, 1024 * (2 * hh + gp) : 1024 * (2 * hh + gp + 1)],
                            psT[:],
                        )

            psA_cm = tc.tile_pool(name="psA", bufs=2, space="PSUM")
            psA = psA_cm.__enter__()

            # stream 2 first (k2T is needed by every attention chunk)
            stream_phase(dx2, x2nT, True)
            # k2T chunks (a-cols 0..2047) + v2 tiles
            for c in range(4):
                ps = psA.tile([128, 512], f32, tag="qk")
                nc.tensor.matmul(
                    ps[:], wk, x2nT[:, 512 * c : 512 * (c + 1)], start=True, stop=True
                )
                nc.scalar.activation(
                    k2T[:, 512 * c : 512 * (c + 1)], ps[:], AF.Identity, bias=bk
                )
            for g in range(NG):
                psv = psA.tile([128, 512], f32, tag="v")
                for k in range(4):
                    i = 4 * g + k
                    nc.tensor.matmul(
                        psv[:, 128 * k : 128 * (k + 1)],
                        x2nT[:, 128 * i : 128 * (i + 1)],
                        wv2,
                        start=True,
                        stop=True,
                    )
                nc.scalar.copy(v2[:, 512 * g : 512 * (g + 1)], psv[:])

            stream_phase(dx1, x1nT, False)
            for c in range(NG):
                ps = psA.tile([128, 512], f32, tag="qk")
                nc.tensor.matmul(
                    ps[:], wq, x1nT[:, 512 * c : 512 * (c + 1)], start=True, stop=True
                )
                nc.scalar.activation(
                    q1T[:, 512 * c : 512 * (c + 1)], ps[:], AF.Identity, bias=bq
                )
            for g in range(NG):
                psv = psA.tile([128, 512], f32, tag="v")
                for k in range(4):
                    i = 4 * g + k
                    nc.tensor.matmul(
                        psv[:, 128 * k : 128 * (k + 1)],
                        x1nT[:, 128 * i : 128 * (i + 1)],
                        wv1,
                        start=True,
                        stop=True,
                    )
                nc.vector.tensor_copy(v1[:, 512 * g : 512 * (g + 1)], psv[:])

            psA_cm.__exit__(None, None, None)

            r1 = respool.tile([128, AT, D], f32, tag="r1")
            nc.sync.dma_start(r1[:], dres1.ap())
            r2 = respool.tile([128, AT, D], f32, tag="r2")
            nc.sync.dma_start(r2[:], dres2.ap())

            # =========== Phase B: attention =============================
            psB_cm = tc.tile_pool(name="psB", bufs=2, space="PSUM")
            psB = psB_cm.__enter__()
            psBo_cm = tc.tile_pool(name="psBo", bufs=1, space="PSUM")
            psBo = psBo_cm.__enter__()
            BSf = stats.tile([128, AT, 6], f32, tag="BSf")
            MVf = stats.tile([128, AT, 2], f32, tag="MVf")
            IVf = stats.tile([128, AT], f32, tag="IVf")
            RSf = stats.tile([128, AT], f32, tag="RSf")
            for j in range(AC):
                psO1 = psBo.tile([128, 512], f32, tag="o1")
                psO2 = psBo.tile([128, 512], f32, tag="o2")
                def av_mms(g):
                    for k in range(2):
                        i = 2 * g + k
                        nc.tensor.matmul(
                            psO1[:],
                            v1[:, 128 * i : 128 * (i + 1)],
                            att_i(i),
                            start=(i == 0),
                            stop=(i == NT - 1),
                            skip_group_check=True,
                        )
                        nc.tensor.matmul(
                            psO2[:],
                            v2[:, 128 * i : 128 * (i + 1)],
                            att_i(i),
                            start=(i == 0),
                            stop=(i == NT - 1),
                            skip_group_check=True,
                        )

                aP = attP[j % 2]
                att_i = lambda i: aP[:, i, :] if i < 4 else attT[:, i, :]
                for g in range(NT // 2):
                    psE = psB.tile([128, 2, 512], f32, tag="e")
                    for k in range(2):
                        i = 2 * g + k
                        nc.tensor.matmul(
                            psE[:, k, :],
                            q1T[:, 128 * i : 128 * (i + 1)],
                            k2T[:, 512 * j : 512 * (j + 1)],
                            start=True,
                            stop=True,
                        )
                    if g < 2:
                        nc.scalar.activation(
                            aP[:, 2 * g : 2 * g + 2, :], psE[:], AF.Exp, scale=SCALE
                        )
                    else:
                        nc.scalar.activation(
                            attT[:, 2 * g : 2 * g + 2, :], psE[:], AF.Exp, scale=SCALE
                        )
                    if g > 0:
                        av_mms(g - 1)
                av_mms(NT // 2 - 1)
                # denominator: pairwise tree into scratch + ones matmuls
                nc.vector.tensor_tensor(
                    T8[:, 0:4, :], aP[:], attT[:, 8:12, :], op=ALU.add
                )
                nc.vector.tensor_tensor(
                    T8[:, 4:8, :], attT[:, 4:8, :], attT[:, 12:16, :], op=ALU.add
                )
                nc.vector.tensor_tensor(T8[:], T8[:], attT[:, 16:24, :], op=ALU.add)
                nc.vector.tensor_tensor(T8[:], T8[:], attT[:, 24:32, :], op=ALU.add)
                w = 8
                while w > 1:
                    w //= 2
                    nc.vector.tensor_tensor(
                        T8[:, 0:w, :], T8[:, 0:w, :], T8[:, w : 2 * w, :], op=ALU.add
                    )
                psDen = psBo.tile([128, 4], f32, tag="den")
                for t in range(4):
                    nc.tensor.matmul(
                        psDen[:, t : t + 1],
                        T8[:, 0, 128 * t : 128 * (t + 1)],
                        ones_bf[:],
                        start=True,
                        stop=True,
                    )
                nc.vector.reciprocal(invd[:, 4 * j : 4 * j + 4], psDen[:])
                nc.scalar.copy(o1T[:, 512 * j : 512 * (j + 1)], psO1[:])
                nc.scalar.copy(o2T[:, 512 * j : 512 * (j + 1)], psO2[:])
                # inline output projection + residual + lnf stats for this chunk
                for k in range(4):
                    t = 4 * j + k
                    for (oT, wp, rr, off) in (
                        (o1T, wp1, r1[:, t, :], 0),
                        (o2T, wp2, r2[:, t, :], D),
                    ):
                        psP = psBo.tile([128, D], f32, tag="p")
                        nc.tensor.matmul(
                            psP[:], oT[:, 128 * t : 128 * (t + 1)], wp,
                            start=True, stop=True,
                        )
                        sc = xspool.tile([128, D], f32, tag="sc")
                        nc.vector.tensor_scalar(
                            sc[:], psP[:], invd[:, t : t + 1], None, op0=ALU.mult
                        )
                        nc.gpsimd.tensor_tensor(
                            xcat[:, t, off : off + D], sc[:], rr, op=ALU.add
                        )
                    nc.vector.bn_stats(BSf[:, t, :], xcat[:, t, :])
                    nc.vector.bn_aggr(MVf[:, t, :], BSf[:, t, :])
                sl = slice(4 * j, 4 * (j + 1))
                nc.vector.reciprocal(IVf[:, sl], MVf[:, sl, 1])

            psBo_cm.__exit__(None, None, None)
            psB_cm.__exit__(None, None, None)

            # =========== Phase D pools ==================================
            psC = ctx.enter_context(tc.tile_pool(name="psC", bufs=2, space="PSUM"))
            psH_pool = ctx.enter_context(tc.tile_pool(name="psH", bufs=1, space="PSUM"))

            # =========== Phase D: FFN + final LN + output ================
            def ln_to_T(src_xcat, dstl, dsth, MVx, RSx, chunks, act_evac=False):
                for jj in chunks:
                    psT2 = psC.tile([128, 4, 256], bf16, tag="tr2")
                    for k in range(4):
                        t = 4 * jj + k
                        xsf = xspool.tile([128, D2], bf16, tag="xsf")
                        nc.vector.tensor_scalar(
                            xsf[:],
                            src_xcat[:, t, :],
                            MVx[:, t, 0:1],
                            RSx[:, t : t + 1],
                            op0=ALU.subtract,
                            op1=ALU.mult,
                        )
                        nc.tensor.transpose(psT2[:, k, 0:128], xsf[:, 0:128], ident[:])
                        nc.tensor.transpose(psT2[:, k, 128:256], xsf[:, 128:256], ident[:])
                    evac = nc.scalar.copy if act_evac else nc.vector.tensor_copy
                    evac(dstl[:, 512 * jj : 512 * (jj + 1)], psT2[:, :, 0:128])
                    evac(dsth[:, 512 * jj : 512 * (jj + 1)], psT2[:, :, 128:256])

            BS3 = stats.tile([128, AT, 6], f32, tag="BS3")
            MV3 = stats.tile([128, AT, 2], f32, tag="MV3")
            IV3 = stats.tile([128, AT], f32, tag="IV3")
            RS3 = stats.tile([128, AT], f32, tag="RS3")
            nc.scalar.activation(RSf[:], IVf[:], AF.Sqrt)
            for jp in range(2):
                ln_to_T(xcat, xfTl, xfTh, MVf, RSf, [2 * jp, 2 * jp + 1])
                for n in range(4):
                    psH = psH_pool.tile([128, 2, 512], f32, tag="h")
                    for jj in range(2):
                        j = 2 * jp + jj
                        nc.tensor.matmul(
                            psH[:, jj, :],
                            wf1v(0, n),
                            xfTl[:, 512 * j : 512 * (j + 1)],
                            start=True,
                            stop=False,
                            skip_group_check=True,
                        )
                        nc.tensor.matmul(
                            psH[:, jj, :],
                            wf1v(1, n),
                            xfTh[:, 512 * j : 512 * (j + 1)],
                            start=False,
                            stop=True,
                            skip_group_check=True,
                        )
                    nc.scalar.activation(
                        h1T[:, n, 1024 * jp : 1024 * (jp + 1)],
                        psH[:],
                        AF.Gelu,
                        bias=bf1t[:, n : n + 1],
                    )
                for t in range(8 * jp, 8 * jp + 8):
                    psH2 = psC.tile([128, D2], f32, tag="h2")
                    for st in range(4):
                        nc.tensor.matmul(
                            psH2[:],
                            h1T[:, st, 128 * t : 128 * (t + 1)],
                            wf2v(st),
                            start=(st == 0),
                            stop=(st == 3),
                            skip_group_check=True,
                        )
                    sc = xspool.tile([128, D2], f32, tag="sc2")
                    nc.vector.tensor_tensor(sc[:], psH2[:], bf2b[:], op=ALU.add)
                    nc.gpsimd.tensor_tensor(
                        xcat[:, t, :], sc[:], xcat[:, t, :], op=ALU.add
                    )
                    nc.vector.bn_stats(BS3[:, t, :], xcat[:, t, :])
                    nc.vector.bn_aggr(MV3[:, t, :], BS3[:, t, :])
                    if t % 4 == 3:
                        nc.vector.reciprocal(
                            IV3[:, t - 3 : t + 1], MV3[:, t - 3 : t + 1, 1]
                        )

            # ln3 (reuse xfT buffers as x3T); sqrt batched to keep the gelu
            # table-set block contiguous
            nc.scalar.activation(RS3[:], IV3[:], AF.Sqrt)
            ov = dout.ap()
            osb = outpool.tile([128, AT, OUT], f32, tag="osb")
            for jj in range(AC):
                ln_to_T(xcat, xfTl, xfTh, MV3, RS3, [jj], act_evac=True)
                for t in range(4 * jj, 4 * jj + 4):
                    psO = psC.tile([128, OUT], f32, tag="p")
                    nc.tensor.matmul(
                        psO[:],
                        xfTl[:, 128 * t : 128 * (t + 1)],
                        wov(0),
                        start=True,
                        stop=False,
                        skip_group_check=True,
                    )
                    nc.tensor.matmul(
                        psO[:],
                        xfTh[:, 128 * t : 128 * (t + 1)],
                        wov(1),
                        start=False,
                        stop=True,
                        skip_group_check=True,
                    )
                    nc.vector.tensor_tensor(osb[:, t, :], psO[:], bob[:], op=ALU.add)
                nc.sync.dma_start(ov[:, 4 * jj : 4 * jj + 4, :], osb[:, 4 * jj : 4 * jj + 4, :])

    nc.compile()
    return nc


def _get_nc():
    if "nc" not in _CACHE:
        _CACHE["nc"] = _build_nc()
    return _CACHE["nc"]


def kernel(**inputs):
    from concourse.bass_utils import run_bass_kernel_spmd

    f = lambda k: np.asarray(inputs[k], dtype=np.float32)
    bf = lambda a: np.asarray(a, dtype=np.float32).astype(ml_dtypes.bfloat16)

    x1, x2 = f("x1"), f("x2")
    g1, b1 = f("ln1_g"), f("ln1_b")
    g2, b2 = f("ln2_g"), f("ln2_b")
    gf_, bf_ = f("lnf_g"), f("lnf_b")
    g3, b3 = f("ln3_g"), f("ln3_b")
    # fold LN gains/biases into the adjacent linear layers
    Wq = g1[:, None] * f("Wq"); bqp = b1 @ f("Wq") + f("bq")
    Wk = g2[:, None] * f("Wk"); bkp = b2 @ f("Wk") + f("bk")
    Wv1 = g1[:, None] * f("Wv1"); bv1p = b1 @ f("Wv1") + f("bv1")
    Wv2 = g2[:, None] * f("Wv2"); bv2p = b2 @ f("Wv2") + f("bv2")
    Wf1 = gf_[:, None] * f("Wf1"); bf1p = bf_ @ f("Wf1") + f("bf1")
    Wo = g3[:, None] * f("Wo"); bop = b3 @ f("Wo") + f("bo")
    Wp1, Wp2 = f("Wp1"), f("Wp2")
    bp1p = bv1p @ Wp1 + f("bp1")
    bp2p = bv2p @ Wp2 + f("bp2")

    Wf2 = f("Wf2")
    wpack = np.concatenate(
        [bf(Wq), bf(Wk), bf(Wv1), bf(Wv2), bf(Wp1), bf(Wp2),
         # Wf1 [256,512] -> [128, 2*4*128] as (kp, kh, n, np)
         bf(Wf1).reshape(2, 128, 4, 128).transpose(1, 0, 2, 3).reshape(128, 1024),
         # Wf2 [512,256] -> [128, 4*256] as (p, s, n)
         bf(Wf2).reshape(4, 128, D2).transpose(1, 0, 2).reshape(128, 1024),
         # Wo [256,55] -> [128, 2*55]
         bf(Wo).reshape(2, 128, OUT).transpose(1, 0, 2).reshape(128, 2 * OUT)],
        axis=1,
    )
    vpack = np.concatenate(
        [bqp.reshape(1, D), bkp.reshape(1, D), bf1p.reshape(4, D)], axis=0
    ).T.astype(np.float32)
    shared = {
        "wpack": np.ascontiguousarray(wpack),
        "vpack": np.ascontiguousarray(vpack),
        "bf2": f("bf2"), "bo": bop,
    }

    in_maps = []
    for c in range(8):
        b, h = c // 2, c % 2
        if h == 0:
            x1c, x2c = x1[b], x2[b]
        else:
            x1c = np.concatenate([x1[b, A:], x1[b, :A]], axis=0)
            x2c = np.concatenate([x2[b, A:], x2[b, :A]], axis=0)
        tilep = lambda M, nt: np.ascontiguousarray(
            M.reshape(nt, 128, D).transpose(1, 0, 2)
        )
        m = dict(shared)
        m["x1"] = tilep(x1c, NT)
        m["x2"] = tilep(x2c, NT)
        m["res1p"] = tilep(x1c[:A] + bp1p, AT)
        m["res2p"] = tilep(x2c[:A] + bp2p, AT)
        in_maps.append(m)

    nc = _get_nc()
    res = run_bass_kernel_spmd(nc, in_maps, core_ids=list(range(8)))
    out = np.empty((B, L, OUT), np.float32)
    for c in range(8):
        b, h = c // 2, c % 2
        oc = res.results[c]["out"].transpose(1, 0, 2).reshape(A, OUT)
        out[b, h * A : (h + 1) * A, :] = oc
    return out
